# revision 10
# baseline (speedup 1.0000x reference)
"""AtomToPair GNN message-passing kernel for 8 TRN2 NeuronCores.

Math (per molecule, A=64 atoms, F=C=128):
    h0[i,j] = MLP([x_i, x_j]),  h1[i,j] = MLP([x_j, x_i]) = h0[j,i]
    out[i,j] = h0[i,j] + h0[j,i]            (symmetric in i,j)

Design (v2):
  * L1 on the PE: Y1pre[:, i, j] = W0top.T x_i + W0bot.T x_j as two
    accumulated bf16 matmuls with broadcast moving-operand APs.
  * L2 computed TWICE but only on the block-upper-triangle: the
    "straight" matmul A = W1.T @ Y1[tri] and the "mirror" matmul
    B = W1.T @ Y1^T[tri], where the transpose is free (it is just a
    transposed access pattern on the moving operand).  Then
        E[tri] = relu(A + b1) + relu(B + b1)
    is two dense step-1 elementwise passes (no strided DVE reads, no
    full-square relu2):  T = relu(A+b1) on ACT;  E = max(B,0)+T as a
    single DVE scalar_tensor_tensor (valid since b1 == 0; a separate
    compiled variant handles b1 != 0).
  * Triangle-packed bf16 output DMA (halves HBM traffic); the host
    upcasts to fp32 and mirror-fills.
  * PSUM budget (8 banks): psY pool 2x[1024] fp32 (4 banks) serves the
    L1 pre-activations AND is reused for the mirror-matmul B tiles;
    psA pool 2x[<=960] (4 banks) holds the straight tiles.

Sharding: data-parallel over batch - each of the 8 cores handles
B/8 = 4 molecules with fully replicated weights.  On-chip layout is
feature-major ([C on partitions, pairs on free]); the host transposes
to the reference layout during the unshard step.
"""

import sys

sys.path.insert(0, "/opt/trn_rl_repo")

import os

import numpy as np

B, A, F, C = 32, 64, 128, 128
NCORES = 8
MPC = B // NCORES          # molecules per core
PAIRS = A * A              # 4096
IB = 8                     # i-block (rows per chunk)
NCHUNK = A // IB           # 8 chunks per molecule
# packed block-triangle: chunk k holds rows i in [8k,8k+8), cols j in [8k,64)
TRI_W = [A - IB * k for k in range(NCHUNK)]      # 64,56,...,8
TRI_OFF = [IB * sum(TRI_W[:k]) for k in range(NCHUNK)]
TRI_COLS = IB * sum(TRI_W)                        # 2304 per molecule

# packed bf16 param columns: w0t | w0b | w1 | xT   (weights first so the
# head DMA [0 : 3C+A) covers everything molecule 0 needs)
W0T_OFF, W0B_OFF, W1_OFF, XB_OFF = 0, C, 2 * C, 3 * C
PB_COLS = 3 * C + MPC * A

_compiled = {}


def _build(safe_bias=False):
    import concourse.bass as bass
    import concourse.tile as tile
    from concourse import bacc, mybir

    fp32 = mybir.dt.float32
    bf16 = mybir.dt.bfloat16
    nc = bacc.Bacc("TRN2", target_bir_lowering=False, debug=False,
                   num_devices=NCORES)

    pb16 = nc.dram_tensor("pb16", [128, PB_COLS], bf16,
                          kind="ExternalInput").ap()
    pf32 = nc.dram_tensor("pf32", [128, 2], fp32, kind="ExternalInput").ap()
    out = nc.dram_tensor("out", [C, MPC * TRI_COLS], bf16,
                         kind="ExternalOutput").ap()

    Relu = mybir.ActivationFunctionType.Relu
    add_op = mybir.AluOpType.add
    max_op = mybir.AluOpType.max

    # relu1 engine per group q (3,2,1,0): True -> DVE, False -> ACT
    r1_dve = os.environ.get("ATOMPAIR_R1DVE", "1100")

    with tile.TileContext(nc) as tc:
        with (
            tc.tile_pool(name="const", bufs=1) as const_pool,
            tc.tile_pool(name="y1", bufs=int(os.environ.get("AP_KY1", "2"))) as y1_pool,
            tc.tile_pool(name="tbuf", bufs=int(os.environ.get("AP_KT", "3"))) as t_pool,
            tc.tile_pool(name="obuf", bufs=int(os.environ.get("AP_KO", "2"))) as o_pool,
            tc.tile_pool(name="psY", bufs=2, space="PSUM") as psY_pool,
            tc.tile_pool(name="psL2", bufs=2, space="PSUM") as psL2_pool,
        ):
            pb = const_pool.tile([128, PB_COLS], bf16, tag="pb")
            pf = const_pool.tile([128, 2], fp32, tag="pf")
            head = 3 * C + A
            nc.sync.dma_start(pb[:, :head], pb16[:, :head])
            nc.sync.dma_start(pb[:, head:], pb16[:, head:])
            nc.sync.dma_start(pf[:], pf32[:])

            w0t_s = pb[:, W0T_OFF: W0T_OFF + C]
            w0b_s = pb[:, W0B_OFF: W0B_OFF + C]
            w1_s = pb[:, W1_OFF: W1_OFF + C]
            b0_s = pf[:, 0:1]
            b1_s = pf[:, 1:2]

            for m in range(MPC):
                xm = pb[:, XB_OFF + m * A: XB_OFF + (m + 1) * A]
                y1m = y1_pool.tile([C, PAIRS], bf16, tag="y1m")
                y3 = y1m[:].rearrange("c (i j) -> c i j", i=A)
                ot = o_pool.tile([C, TRI_COLS], bf16, tag="ot")

                # ---- L1 for the whole molecule, reverse chunk-pair
                # order; w0t for both chunks, then w0b (2 LDW total) ----
                rhs_j = xm.unsqueeze(1).to_broadcast((F, IB, A))
                for q in reversed(range(NCHUNK // 2)):
                    ka, kb = 2 * q + 1, 2 * q
                    psy = psY_pool.tile([C, 2 * IB * A], fp32, tag="psY")
                    for h, k in enumerate((kb, ka)):
                        xi = xm[:, k * IB: (k + 1) * IB]
                        rhs_i = xi.unsqueeze(2).to_broadcast((F, IB, A))
                        ps3 = psy[:, h * IB * A: (h + 1) * IB * A].rearrange(
                            "c (i j) -> c i j", i=IB)
                        nc.tensor.matmul(ps3, w0t_s, rhs_i,
                                         start=True, stop=False)
                    for h, k in enumerate((kb, ka)):
                        ps3 = psy[:, h * IB * A: (h + 1) * IB * A].rearrange(
                            "c (i j) -> c i j", i=IB)
                        nc.tensor.matmul(ps3, w0b_s, rhs_j,
                                         start=False, stop=True)

                    # ---- relu1 (+b0) -> bf16 Y1 chunks ----
                    ydst = y1m[:, kb * IB * A: (kb + 2) * IB * A]
                    if r1_dve[3 - q] == "1":
                        nc.vector.tensor_scalar(ydst, psy[:], b0_s, 0.0,
                                                add_op, max_op)
                    else:
                        nc.scalar.activation(ydst, psy[:], Relu, bias=b0_s)

                # ---- L2: reverse order mid-stream (B deps ready as soon
                # as each group's relu1 lands); forward order for the
                # last molecule so the kernel tail is the smallest group.
                qorder = (range(NCHUNK // 2) if m == MPC - 1
                          else reversed(range(NCHUNK // 2)))
                for q in qorder:
                    ka, kb = 2 * q + 1, 2 * q
                    wa, wb = TRI_W[ka], TRI_W[kb]
                    gw = IB * (wa + wb)
                    # PSUM matmul tiles must not straddle a 2KB bank:
                    # chunk kb lives at [0, 8*wb), chunk ka at [512, 512+8*wa)
                    psa = psL2_pool.tile([C, 2 * IB * A], fp32, tag="psL2")
                    psb = psL2_pool.tile([C, 2 * IB * A], fp32, tag="psL2")
                    for h, k in enumerate((kb, ka)):
                        w = TRI_W[k]
                        off = h * IB * A
                        straight = y3[:, k * IB: (k + 1) * IB, k * IB:]
                        pa3 = psa[:, off: off + IB * w].rearrange(
                            "c (i j) -> c i j", i=IB)
                        nc.tensor.matmul(pa3, w1_s, straight,
                                         start=True, stop=True)
                        mirror = y3[:, k * IB:, k * IB: (k + 1) * IB]
                        mirror = mirror.transpose([0, 2, 1])
                        pb3 = psb[:, off: off + IB * w].rearrange(
                            "c (i j) -> c i j", i=IB)
                        nc.tensor.matmul(pb3, w1_s, mirror,
                                         start=True, stop=True)

                    # ---- E[tri] = relu(A+b1) + relu(B+b1) ----
                    # reluA covers [0, 512+8*wa) in one pass (the pad
                    # between the chunks is harmless garbage); the stt
                    # adds run per chunk so E stays triangle-packed.
                    tsb = t_pool.tile([C, 2 * IB * A], bf16, tag="tsb")
                    aw = IB * A + IB * wa
                    nc.scalar.activation(tsb[:, :aw], psa[:, :aw], Relu,
                                         bias=b1_s)
                    ob = out[:, m * TRI_COLS: (m + 1) * TRI_COLS]
                    for h, k in enumerate((kb, ka)):
                        w = TRI_W[k]
                        off = h * IB * A
                        eseg = ot[:, TRI_OFF[k]: TRI_OFF[k] + IB * w]
                        if safe_bias:
                            t2 = t_pool.tile([C, 2 * IB * A], bf16,
                                             tag="t2")
                            nc.scalar.activation(t2[:, off: off + IB * w],
                                                 psb[:, off: off + IB * w],
                                                 Relu, bias=b1_s)
                            nc.vector.tensor_tensor(
                                eseg, tsb[:, off: off + IB * w],
                                t2[:, off: off + IB * w], add_op)
                        else:
                            nc.vector.scalar_tensor_tensor(
                                eseg, psb[:, off: off + IB * w], 0.0,
                                tsb[:, off: off + IB * w],
                                op0=max_op, op1=add_op)
                    nc.sync.dma_start(ob[:, TRI_OFF[kb]: TRI_OFF[kb] + gw],
                                      ot[:, TRI_OFF[kb]: TRI_OFF[kb] + gw])
    nc.compile()
    return nc


def _get_compiled(safe_bias=False, fused=False):
    key = bool(safe_bias)
    if key not in _compiled:
        _compiled[key] = _build(safe_bias=key)
    return _compiled[key]


def _shard_inputs(x, W0, b0, W1, b1):
    import ml_dtypes

    bf = ml_dtypes.bfloat16
    pf32 = np.stack([b0, b1], axis=1).astype(np.float32)  # [128, 2]
    w_cols = np.concatenate([W0[:F], W0[F:], W1], axis=1).astype(bf)
    in_maps = []
    for c in range(NCORES):
        xs = x[c * MPC: (c + 1) * MPC]                    # [MPC, A, F]
        xTs = xs.transpose(2, 0, 1).reshape(F, MPC * A)
        pb16 = np.ascontiguousarray(
            np.concatenate([w_cols, xTs.astype(bf)], axis=1))
        in_maps.append({"pb16": pb16, "pf32": pf32})
    return in_maps


def _unshard(results):
    """[C, MPC*TRI_COLS] bf16 per core -> full (B, A*A, C) fp32 w/ mirror."""
    full = np.empty((B, A, A, C), dtype=np.float32)
    for c in range(NCORES):
        o = np.asarray(results[c]["out"]).astype(np.float32)
        for m in range(MPC):
            bidx = c * MPC + m
            pk = o[:, m * TRI_COLS: (m + 1) * TRI_COLS]
            for k in range(NCHUNK):
                w = TRI_W[k]
                blk = pk[:, TRI_OFF[k]: TRI_OFF[k] + IB * w]
                blk = blk.reshape(C, IB, w).transpose(1, 2, 0)
                full[bidx, k * IB: (k + 1) * IB, k * IB:] = blk
                if k > 0:
                    full[bidx, k * IB: (k + 1) * IB, : k * IB] = \
                        full[bidx, : k * IB, k * IB: (k + 1) * IB] \
                        .transpose(1, 0, 2)
    return full.reshape(B, A * A, C)


def kernel(x, W0, b0, W1, b1):
    from concourse.bass_utils import run_bass_kernel_spmd

    x = np.asarray(x, dtype=np.float32)
    W0 = np.asarray(W0, dtype=np.float32)
    b0 = np.asarray(b0, dtype=np.float32)
    W1 = np.asarray(W1, dtype=np.float32)
    b1 = np.asarray(b1, dtype=np.float32)

    in_maps = _shard_inputs(x, W0, b0, W1, b1)
    nc = _get_compiled(safe_bias=bool(np.any(b1)))
    res = run_bass_kernel_spmd(nc, in_maps, core_ids=list(range(NCORES)))
    return _unshard(res.results)


# revision 12
# speedup vs baseline: 1.0581x; 1.0581x over previous
"""AtomToPair GNN message-passing kernel for 8 TRN2 NeuronCores.

Math (per molecule, A=64 atoms, F=C=128):
    h0[i,j] = MLP([x_i, x_j]),  h1[i,j] = MLP([x_j, x_i]) = h0[j,i]
    out[i,j] = h0[i,j] + h0[j,i]            (symmetric in i,j)

Design (v4):
  * L1 on the PE: Y1pre[:, i, j] = W0top.T x_i + W0bot.T x_j as two
    accumulated bf16 matmuls with broadcast moving-operand APs.
  * L2 computed TWICE but only on the block-upper-triangle: the
    "straight" matmul A = W1.T @ Y1[tri] and the "mirror" matmul
    B = W1.T @ Y1^T[tri], where the transpose is free (a transposed
    access pattern on the moving operand).  Then
        E[tri] = relu(A + b1) + relu(B + b1)
    is dense step-1 elementwise work only:  T = relu(A+b1) on ACT;
    E = max(B,0)+T as a DVE scalar_tensor_tensor per chunk (valid
    since b1 == 0; a separate compiled variant handles b1 != 0).
  * Flat software-pipelined schedule over 16 (molecule, chunk-pair)
    groups with a one-group skew between L1 and L2 so the PE never
    waits on relu1; chunk-pair order is reversed (7,6 .. 1,0) per
    molecule because the mirror matmul of chunk k reads Y1 rows >= k.
  * Triangle-packed bf16 output DMA (halves HBM traffic); the host
    upcasts to fp32 and mirror-fills.
  * PSUM (8 banks): psY 2x[1024] fp32 for L1, psL2 2x[1024] for the
    A/B tiles, each chunk's matmul output bank-aligned (kb at col 0,
    ka at col 512 of its tile).

Sharding: data-parallel over batch - each of the 8 cores handles
B/8 = 4 molecules with fully replicated weights.  On-chip layout is
feature-major ([C on partitions, pairs on free]); the host transposes
to the reference layout during the unshard step.
"""

import sys

sys.path.insert(0, "/opt/trn_rl_repo")

import os

import numpy as np

B, A, F, C = 32, 64, 128, 128
NCORES = 8
MPC = B // NCORES          # molecules per core
PAIRS = A * A              # 4096
IB = 8                     # i-block (rows per chunk)
NCHUNK = A // IB           # 8 chunks per molecule
NG = NCHUNK // 2           # chunk-pair groups per molecule
# packed block-triangle: chunk k holds rows i in [8k,8k+8), cols j in [8k,64)
TRI_W = [A - IB * k for k in range(NCHUNK)]      # 64,56,...,8
TRI_OFF = [IB * sum(TRI_W[:k]) for k in range(NCHUNK)]
TRI_COLS = IB * sum(TRI_W)                        # 2304 per molecule

# packed bf16 param columns: w0t | w0b | w1 | xT   (weights first so the
# head DMA [0 : 3C+A) covers everything molecule 0 needs)
W0T_OFF, W0B_OFF, W1_OFF, XB_OFF = 0, C, 2 * C, 3 * C
PB_COLS = 3 * C + MPC * A

_compiled = {}


def _build(safe_bias=False):
    import concourse.bass as bass
    import concourse.tile as tile
    from concourse import bacc, mybir

    fp32 = mybir.dt.float32
    bf16 = mybir.dt.bfloat16
    nc = bacc.Bacc("TRN2", target_bir_lowering=False, debug=False,
                   num_devices=NCORES)

    pb16 = nc.dram_tensor("pb16", [128, PB_COLS], bf16,
                          kind="ExternalInput").ap()
    pf32 = nc.dram_tensor("pf32", [128, 2], fp32, kind="ExternalInput").ap()
    out = nc.dram_tensor("out", [C, MPC * TRI_COLS], bf16,
                         kind="ExternalOutput").ap()

    Relu = mybir.ActivationFunctionType.Relu
    add_op = mybir.AluOpType.add
    max_op = mybir.AluOpType.max

    # relu1 engine per group q (3,2,1,0): 1 -> DVE, 0 -> ACT
    r1_dve = os.environ.get("ATOMPAIR_R1DVE", "1100")

    with tile.TileContext(nc) as tc:
        with (
            tc.tile_pool(name="const", bufs=1) as const_pool,
            tc.tile_pool(name="y1", bufs=int(os.environ.get("AP_KY1", "2"))) as y1_pool,
            tc.tile_pool(name="tbuf", bufs=int(os.environ.get("AP_KT", "3"))) as t_pool,
            tc.tile_pool(name="obuf", bufs=int(os.environ.get("AP_KO", "2"))) as o_pool,
            tc.tile_pool(name="psY", bufs=2, space="PSUM") as psY_pool,
            tc.tile_pool(name="psL2", bufs=2, space="PSUM") as psL2_pool,
        ):
            pb = const_pool.tile([128, PB_COLS], bf16, tag="pb")
            pf = const_pool.tile([128, 2], fp32, tag="pf")
            head = 3 * C + A
            nc.sync.dma_start(pb[:, :head], pb16[:, :head])
            nc.sync.dma_start(pb[:, head:], pb16[:, head:])
            nc.sync.dma_start(pf[:], pf32[:])

            w0t_s = pb[:, W0T_OFF: W0T_OFF + C]
            w0b_s = pb[:, W0B_OFF: W0B_OFF + C]
            w1_s = pb[:, W1_OFF: W1_OFF + C]
            b0_s = pf[:, 0:1]
            b1_s = pf[:, 1:2]

            y1ms = {}
            ots = {}

            def emit_L1(m, q):
                if m not in y1ms or q == NG - 1:
                    y1ms[m] = y1_pool.tile([C, PAIRS], bf16, tag="y1m", name="y1m")
                    ots[m] = o_pool.tile([C, TRI_COLS], bf16, tag="ot", name="ot")
                xm = pb[:, XB_OFF + m * A: XB_OFF + (m + 1) * A]
                ka, kb = 2 * q + 1, 2 * q
                psy = psY_pool.tile([C, 2 * IB * A], fp32, tag="psY")
                for h, k in enumerate((kb, ka)):
                    xi = xm[:, k * IB: (k + 1) * IB]
                    rhs_i = xi.unsqueeze(2).to_broadcast((F, IB, A))
                    ps3 = psy[:, h * IB * A: (h + 1) * IB * A].rearrange(
                        "c (i j) -> c i j", i=IB)
                    nc.tensor.matmul(ps3, w0t_s, rhs_i,
                                     start=True, stop=False)
                rhs_j = xm.unsqueeze(1).to_broadcast((F, IB, A))
                for h, k in enumerate((kb, ka)):
                    ps3 = psy[:, h * IB * A: (h + 1) * IB * A].rearrange(
                        "c (i j) -> c i j", i=IB)
                    nc.tensor.matmul(ps3, w0b_s, rhs_j,
                                     start=False, stop=True)
                # relu1 (+b0) -> bf16 Y1 chunks
                ydst = y1ms[m][:, kb * IB * A: (kb + 2) * IB * A]
                if r1_dve[NG - 1 - q] == "1":
                    nc.vector.tensor_scalar(ydst, psy[:], b0_s, 0.0,
                                            add_op, max_op)
                else:
                    nc.scalar.activation(ydst, psy[:], Relu, bias=b0_s)

            def emit_L2(m, q):
                y3 = y1ms[m][:].rearrange("c (i j) -> c i j", i=A)
                ot = ots[m]
                ka, kb = 2 * q + 1, 2 * q
                wa, wb = TRI_W[ka], TRI_W[kb]
                gw = IB * (wa + wb)
                # PSUM matmul tiles must not straddle a 2KB bank:
                # chunk kb lives at [0, 8*wb), chunk ka at [512, 512+8*wa)
                psa = psL2_pool.tile([C, 2 * IB * A], fp32, tag="psL2")
                psb = psL2_pool.tile([C, 2 * IB * A], fp32, tag="psL2")
                for h, k in enumerate((kb, ka)):
                    w = TRI_W[k]
                    off = h * IB * A
                    straight = y3[:, k * IB: (k + 1) * IB, k * IB:]
                    pa3 = psa[:, off: off + IB * w].rearrange(
                        "c (i j) -> c i j", i=IB)
                    nc.tensor.matmul(pa3, w1_s, straight,
                                     start=True, stop=True)
                    mirror = y3[:, k * IB:, k * IB: (k + 1) * IB]
                    mirror = mirror.transpose([0, 2, 1])
                    pb3 = psb[:, off: off + IB * w].rearrange(
                        "c (i j) -> c i j", i=IB)
                    nc.tensor.matmul(pb3, w1_s, mirror,
                                     start=True, stop=True)

                # E[tri] = relu(A+b1) + relu(B+b1).  reluA covers
                # [0, 512+8*wa) in one pass (the pad between the chunks
                # is harmless garbage); the stt adds run per chunk so E
                # stays triangle-packed.
                tsb = t_pool.tile([C, 2 * IB * A], bf16, tag="tsb")
                aw = IB * A + IB * wa
                nc.scalar.activation(tsb[:, :aw], psa[:, :aw], Relu,
                                     bias=b1_s)
                ob = out[:, m * TRI_COLS: (m + 1) * TRI_COLS]
                for h, k in enumerate((kb, ka)):
                    w = TRI_W[k]
                    off = h * IB * A
                    eseg = ot[:, TRI_OFF[k]: TRI_OFF[k] + IB * w]
                    if safe_bias:
                        t2 = t_pool.tile([C, 2 * IB * A], bf16, tag="t2")
                        nc.scalar.activation(t2[:, off: off + IB * w],
                                             psb[:, off: off + IB * w],
                                             Relu, bias=b1_s)
                        nc.vector.tensor_tensor(
                            eseg, tsb[:, off: off + IB * w],
                            t2[:, off: off + IB * w], add_op)
                    else:
                        nc.vector.scalar_tensor_tensor(
                            eseg, psb[:, off: off + IB * w], 0.0,
                            tsb[:, off: off + IB * w],
                            op0=max_op, op1=add_op)
                nc.sync.dma_start(ob[:, TRI_OFF[kb]: TRI_OFF[kb] + gw],
                                  ot[:, TRI_OFF[kb]: TRI_OFF[kb] + gw])

            # flat schedule with one-group L1->L2 skew
            groups = [(m, q) for m in range(MPC)
                      for q in reversed(range(NG))]
            for i, (m, q) in enumerate(groups):
                emit_L1(m, q)
                if i > 0:
                    emit_L2(*groups[i - 1])
            emit_L2(*groups[-1])
    nc.compile()
    return nc


def _get_compiled(safe_bias=False, fused=False):
    key = bool(safe_bias)
    if key not in _compiled:
        _compiled[key] = _build(safe_bias=key)
    return _compiled[key]


def _shard_inputs(x, W0, b0, W1, b1):
    import ml_dtypes

    bf = ml_dtypes.bfloat16
    pf32 = np.stack([b0, b1], axis=1).astype(np.float32)  # [128, 2]
    w_cols = np.concatenate([W0[:F], W0[F:], W1], axis=1).astype(bf)
    in_maps = []
    for c in range(NCORES):
        xs = x[c * MPC: (c + 1) * MPC]                    # [MPC, A, F]
        xTs = xs.transpose(2, 0, 1).reshape(F, MPC * A)
        pb16 = np.ascontiguousarray(
            np.concatenate([w_cols, xTs.astype(bf)], axis=1))
        in_maps.append({"pb16": pb16, "pf32": pf32})
    return in_maps


def _unshard(results):
    """[C, MPC*TRI_COLS] bf16 per core -> full (B, A*A, C) fp32 w/ mirror."""
    full = np.empty((B, A, A, C), dtype=np.float32)
    for c in range(NCORES):
        o = np.asarray(results[c]["out"]).astype(np.float32)
        for m in range(MPC):
            bidx = c * MPC + m
            pk = o[:, m * TRI_COLS: (m + 1) * TRI_COLS]
            for k in range(NCHUNK):
                w = TRI_W[k]
                blk = pk[:, TRI_OFF[k]: TRI_OFF[k] + IB * w]
                blk = blk.reshape(C, IB, w).transpose(1, 2, 0)
                full[bidx, k * IB: (k + 1) * IB, k * IB:] = blk
                if k > 0:
                    full[bidx, k * IB: (k + 1) * IB, : k * IB] = \
                        full[bidx, : k * IB, k * IB: (k + 1) * IB] \
                        .transpose(1, 0, 2)
    return full.reshape(B, A * A, C)


def kernel(x, W0, b0, W1, b1):
    from concourse.bass_utils import run_bass_kernel_spmd

    x = np.asarray(x, dtype=np.float32)
    W0 = np.asarray(W0, dtype=np.float32)
    b0 = np.asarray(b0, dtype=np.float32)
    W1 = np.asarray(W1, dtype=np.float32)
    b1 = np.asarray(b1, dtype=np.float32)

    in_maps = _shard_inputs(x, W0, b0, W1, b1)
    nc = _get_compiled(safe_bias=bool(np.any(b1)))
    res = run_bass_kernel_spmd(nc, in_maps, core_ids=list(range(NCORES)))
    return _unshard(res.results)
